# revision 2
# baseline (speedup 1.0000x reference)
"""GNN GraphConv x2 + Linear on 8 TRN2 cores.

Strategy (graph/data parallel, per sharding hint):
- Host: bin-pack nodes into buckets of <=32 node slots / <=512 edge slots
  (edges bucketed by dst).  Buckets are dealt to cores; each core owns NB
  buckets.  PSUM groups of GB=8 buckets aggregate 256 node slots at a time.
- dma_gather provides the per-edge source-row gather, but its indices are
  int16, so the source rows are split into K=4 windows of <=32k rows.  Chunk
  (bucket u, window k) holds only edges whose gather key (src row / permuted
  src slot) lies in window k; per-group overflow chunks (one per window, with
  a group-wide 256-slot one-hot) absorb bucket-window overflow so capacity
  constraints stay loose.  Columns are laid out so one dma_gather call covers
  all window-k chunks of a 5-group super-group.
- Device, per layer: dma_gather rows into [128, C, 64] chunks; one-hot
  selection matrices S[e, j] = (dst_local[e] == j) built with iota+is_equal
  on DVE; aggregation agg_T[f, j] += Xg.T @ S on TensorE into PSUM; then
  W_rel @ agg_T + W_root @ x_T, bias+relu on ScalarE -- all feature-major
  (transposed) so no transposes sit in the main path.  Between layers, h1
  rows are transposed back (TensorE), written to HBM and exchanged with an
  AllGather so every core can gather any src row.
- Output is produced feature-major [3, SLOTS] per core; host inverse-permutes.
"""

import numpy as np

import concourse.bacc as bacc
import concourse.bass as bass
import concourse.tile as tile
from concourse import mybir
from concourse.masks import make_identity

P = 128           # partitions / edge-chunk size
D = 64            # feature dim
BN = 32           # node slots per bucket
K = 4             # chunks per bucket = gather windows
GB = 8            # buckets per PSUM group
SUPER = 5         # groups per gather super-group
GSLOT = GB * BN   # 256 node slots per group
SGR = SUPER * GB  # regular chunk cols per (super-group, window) = 40
NOV = 2           # overflow chunks per (super-group, window)
SGA = SGR + NOV   # cols per (super-group, window) incl overflow = 42
PAD_DSTL = 99999.0  # dst_local for padding edge slots (matches no iota slot)

F32 = mybir.dt.float32
I16 = mybir.dt.int16


class Cfg:
    def __init__(self, n_nodes, n_cores, nb_per_core):
        self.n_nodes = n_nodes
        self.n_cores = n_cores
        self.nb = nb_per_core                 # buckets per core
        assert self.nb % (GB * SUPER) == 0
        self.slots = self.nb * BN             # node slots per core
        self.groups = self.nb // GB
        self.supers = self.groups // SUPER
        self.ch = K * (self.nb + self.groups)  # chunk cols per core
        self.gslots = self.n_cores * self.slots

    def windows(self, n_src):
        span = -(-n_src // K)
        assert span <= 32768, (n_src, span)
        return span, [(k * span, min(span, n_src - k * span)) for k in range(K)]


# ---------------------------------------------------------------- host side

def _pack_buckets(deg, nb_total):
    """Assign each node to a bucket (<=BN nodes, <=K*P edges)."""
    import heapq
    n = deg.shape[0]
    order = np.argsort(-deg, kind="stable")
    bucket_of = np.empty(n, np.int64)
    slot_of = np.empty(n, np.int64)
    nodes_in = np.zeros(nb_total, np.int64)
    edges_in = np.zeros(nb_total, np.int64)
    heap = [(0, b) for b in range(nb_total)]
    heapq.heapify(heap)
    cap_e = K * P
    for v in order:
        d = int(deg[v])
        while True:
            if not heap:
                return None
            e, b = heapq.heappop(heap)
            if nodes_in[b] < BN:
                break
        if e + d > cap_e:
            return None
        bucket_of[v] = b
        slot_of[v] = nodes_in[b]
        nodes_in[b] += 1
        edges_in[b] += d
        if nodes_in[b] < BN:
            heapq.heappush(heap, (edges_in[b], b))
    return bucket_of, slot_of, edges_in


def prepare(x, edge_index, W1_rel, b1_rel, W1_root, W2_rel, b2_rel, W2_root,
            W_lin, b_lin, n_cores=8):
    """Host preprocessing: returns (cfg, in_maps, meta)."""
    n_nodes = x.shape[0]
    src = np.asarray(edge_index[0], np.int64)
    dst = np.asarray(edge_index[1], np.int64)
    n_edges = src.shape[0]
    deg = np.bincount(dst, minlength=n_nodes)

    unit = GB * SUPER
    nb = unit
    while nb * n_cores * BN < n_nodes or nb * n_cores * K * P < n_edges:
        nb += unit
    while True:
        res = _pack_buckets(deg, nb * n_cores)
        if res is not None:
            break
        nb += unit
    bucket_of, slot_in_bucket, edges_in = res
    cfg = Cfg(n_nodes, n_cores, nb)
    nb_total = nb * n_cores

    # deal buckets to cores, serpentine by load for edge balance
    border = np.argsort(-edges_in, kind="stable")
    core_of_bucket = np.empty(nb_total, np.int64)
    local_of_bucket = np.empty(nb_total, np.int64)
    for i, b in enumerate(border):
        rnd, pos = divmod(i, n_cores)
        c = pos if rnd % 2 == 0 else n_cores - 1 - pos
        core_of_bucket[b] = c
        local_of_bucket[b] = rnd

    core_of_node = core_of_bucket[bucket_of]
    slot_of_node = local_of_bucket[bucket_of] * BN + slot_in_bucket
    gslot_of_node = core_of_node * cfg.slots + slot_of_node

    span1, wins1 = cfg.windows(n_nodes)
    span2, wins2 = cfg.windows(cfg.gslots)

    # group edges per (core, local bucket)
    e_core = core_of_node[dst]
    e_lb = local_of_bucket[bucket_of[dst]]
    e_dstl = slot_in_bucket[dst].astype(np.float32)
    ekey = e_core * nb + e_lb
    eorder = np.argsort(ekey, kind="stable")
    starts = np.searchsorted(ekey[eorder], np.arange(nb_total + 1))
    ecore_s = e_core[eorder]
    elb_s = e_lb[eorder]
    dstl_s = e_dstl[eorder]

    # column index within a core: c = sg*(K*SGA) + k*SGA + j
    #   j in [0, SGR): regular chunk of bucket  u = sg*SGR + j
    #   j in [SGR, SGA): overflow chunk of group g = sg*SUPER + (j - SGR)
    def col_regular(u, k):
        sg, j = divmod(u, SGR)
        return sg * K * SGA + k * SGA + j

    gidx = np.zeros((2, n_cores, P, cfg.ch), np.int16)
    dstl = np.full((2, n_cores, P, cfg.ch), PAD_DSTL, np.float32)
    from collections import defaultdict
    for li, (keys, span) in enumerate(
            [(src, span1), (gslot_of_node[src], span2)]):
        kv = keys[eorder]
        oflow = defaultdict(list)   # (core, sg, k) -> [(rel_idx, sg_slot)]
        for u in range(nb_total):
            lo, hi = starts[u], starts[u + 1]
            if lo == hi:
                continue
            kk = kv[lo:hi]
            dd = dstl_s[lo:hi]
            cc = ecore_s[lo]
            ub = elb_s[lo]
            sg = ub // SGR
            sg_base = (ub % SGR) * BN
            kw = np.minimum(kk // span, K - 1)
            for k in range(K):
                pos = np.nonzero(kw == k)[0]
                main, over = pos[:P], pos[P:]
                if len(main):
                    col = col_regular(ub, k)
                    lanes = np.arange(len(main))
                    gidx[li, cc, lanes, col] = (kk[main] - k * span).astype(np.int16)
                    dstl[li, cc, lanes, col] = dd[main]
                for p in over:
                    oflow[(cc, sg, k)].append(
                        (int(kk[p] - k * span), float(sg_base + dd[p])))
        for (cc, sg, k), lst in oflow.items():
            assert len(lst) <= NOV * P, "super-group overflow chunks full"
            for i, (ri, sl) in enumerate(lst):
                o, lane = divmod(i, P)
                col = sg * K * SGA + k * SGA + SGR + o
                gidx[li, cc, lane, col] = ri
                dstl[li, cc, lane, col] = sl

    # wrap gather indices into the [16, num/16] call layout (replicated
    # across the 8 gpsimd cores' partition groups).
    # call (sg, k): chunk cols sg*K*SGA + k*SGA + [0, SGA) -> SGA*P idxs
    cw = SGA * P // 16
    gwrap = np.zeros((2, n_cores, P, cfg.supers * K * cw), np.int16)
    for li in range(2):
        for sg in range(cfg.supers):
            for k in range(K):
                c0 = sg * K * SGA + k * SGA
                cols = gidx[li, :, :, c0:c0 + SGA]            # [C, P, SGA]
                vals = cols.transpose(0, 2, 1).reshape(n_cores, -1)
                blk = vals.reshape(n_cores, -1, 16).transpose(0, 2, 1)
                ci = (sg * K + k) * cw
                for rep in range(P // 16):
                    gwrap[li, :, rep * 16:(rep + 1) * 16, ci:ci + cw] = blk

    # x rows per slot, transposed, per core
    xpermT = np.zeros((n_cores, D, cfg.slots), np.float32)
    xpermT[core_of_node, :, slot_of_node] = np.asarray(x, np.float32)

    x_np = np.ascontiguousarray(np.asarray(x, np.float32))
    common = {
        "xfull": x_np,
        "w1relT": np.ascontiguousarray(np.asarray(W1_rel, np.float32).T),
        "w1rootT": np.ascontiguousarray(np.asarray(W1_root, np.float32).T),
        "w2relT": np.ascontiguousarray(np.asarray(W2_rel, np.float32).T),
        "w2rootT": np.ascontiguousarray(np.asarray(W2_root, np.float32).T),
        "wlinT": np.ascontiguousarray(np.asarray(W_lin, np.float32).T),
        "b1": np.asarray(b1_rel, np.float32).reshape(D, 1).copy(),
        "b2": np.asarray(b2_rel, np.float32).reshape(D, 1).copy(),
        "blin": np.asarray(b_lin, np.float32).reshape(3, 1).copy(),
    }
    in_maps = []
    for c in range(n_cores):
        m = dict(common)
        m["gidx1"] = np.ascontiguousarray(gwrap[0, c])
        m["gidx2"] = np.ascontiguousarray(gwrap[1, c])
        m["dstl1"] = np.ascontiguousarray(dstl[0, c])
        m["dstl2"] = np.ascontiguousarray(dstl[1, c])
        m["xpermT"] = np.ascontiguousarray(xpermT[c])
        in_maps.append(m)

    meta = (core_of_node, slot_of_node)
    return cfg, in_maps, meta


def unshard(results, cfg, meta):
    core_of_node, slot_of_node = meta
    outT = np.stack([results[c]["outT"] for c in range(cfg.n_cores)])
    return np.ascontiguousarray(outT[core_of_node, :, slot_of_node])


# -------------------------------------------------------------- device side

def build_program(cfg, debug_dump=False, only_gather=False, skip_collective=False, skip_gather=False, repeat=1):
    nc = bacc.Bacc("TRN2", target_bir_lowering=False, debug=False,
                   num_devices=cfg.n_cores)
    f = F32
    SGCH = K * SGA          # chunk cols per super-group = 180
    NIDX = SGA * P          # idxs per gather call = 5760
    CW = NIDX // 16         # idx cols per call = 360
    NW = cfg.supers * K * CW
    xfull = nc.dram_tensor("xfull", [cfg.n_nodes, D], f, kind="ExternalInput")
    gidx1 = nc.dram_tensor("gidx1", [P, NW], I16, kind="ExternalInput")
    gidx2 = nc.dram_tensor("gidx2", [P, NW], I16, kind="ExternalInput")
    dstl1 = nc.dram_tensor("dstl1", [P, cfg.ch], f, kind="ExternalInput")
    dstl2 = nc.dram_tensor("dstl2", [P, cfg.ch], f, kind="ExternalInput")
    xpermT = nc.dram_tensor("xpermT", [D, cfg.slots], f, kind="ExternalInput")
    w1relT = nc.dram_tensor("w1relT", [D, D], f, kind="ExternalInput")
    w1rootT = nc.dram_tensor("w1rootT", [D, D], f, kind="ExternalInput")
    w2relT = nc.dram_tensor("w2relT", [D, D], f, kind="ExternalInput")
    w2rootT = nc.dram_tensor("w2rootT", [D, D], f, kind="ExternalInput")
    wlinT = nc.dram_tensor("wlinT", [D, 3], f, kind="ExternalInput")
    b1 = nc.dram_tensor("b1", [D, 1], f, kind="ExternalInput")
    b2 = nc.dram_tensor("b2", [D, 1], f, kind="ExternalInput")
    blin = nc.dram_tensor("blin", [3, 1], f, kind="ExternalInput")
    outT = nc.dram_tensor("outT", [3, cfg.slots], f, kind="ExternalOutput")

    h1own = nc.dram_tensor("h1own", [cfg.slots, D], f)
    h1ownT = nc.dram_tensor("h1ownT", [D, cfg.slots], f)
    h1all = nc.dram_tensor("h1all", [cfg.gslots, D], f, addr_space="Shared")
    if debug_dump:
        xgdbg = nc.dram_tensor("xgdbg", [P, SGCH, D], f, kind="ExternalOutput")
        aggdbg = nc.dram_tensor("aggdbg", [D, GSLOT], f, kind="ExternalOutput")
        h1dbg = nc.dram_tensor("h1dbg", [cfg.slots, D], f,
                               kind="ExternalOutput")
        h1alldbg = nc.dram_tensor("h1alldbg", [cfg.gslots, D], f,
                                  kind="ExternalOutput")

    Relu = mybir.ActivationFunctionType.Relu
    _, wins1 = cfg.windows(cfg.n_nodes)
    _, wins2 = cfg.windows(cfg.gslots)

    with tile.TileContext(nc) as tc:
        with (
            tc.tile_pool(name="static", bufs=1) as st_pool,
            tc.tile_pool(name="gst", bufs=2) as gst_pool,
            tc.tile_pool(name="xg", bufs=2) as xg_pool,
            tc.tile_pool(name="selr", bufs=2) as selr_pool,
            tc.tile_pool(name="selo", bufs=2) as selo_pool,
            tc.tile_pool(name="drain", bufs=2) as dr_pool,
            tc.tile_pool(name="root", bufs=2) as root_pool,
            tc.tile_pool(name="outs", bufs=2) as out_pool,
            tc.tile_pool(name="pagg", bufs=2, space="PSUM") as pagg_pool,
            tc.tile_pool(name="ph", bufs=2, space="PSUM") as ph_pool,
            tc.tile_pool(name="pmisc", bufs=2, space="PSUM") as pmisc_pool,
        ):
            def load(name, dram, shape, dtype=f):
                t = st_pool.tile(shape, dtype, name=name)
                nc.sync.dma_start(out=t[:], in_=dram[:])
                return t

            sb_w1relT = load("sb_w1relT", w1relT, [D, D])
            sb_w1rootT = load("sb_w1rootT", w1rootT, [D, D])
            sb_w2relT = load("sb_w2relT", w2relT, [D, D])
            sb_w2rootT = load("sb_w2rootT", w2rootT, [D, D])
            sb_wlinT = load("sb_wlinT", wlinT, [D, 3])
            sb_b1 = load("sb_b1", b1, [D, 1])
            sb_b2 = load("sb_b2", b2, [D, 1])
            sb_blin = load("sb_blin", blin, [3, 1])

            sb_iota = st_pool.tile([P, SUPER * GSLOT], f, name="sb_iota")
            nc.gpsimd.iota(sb_iota[:], pattern=[[1, SUPER * GSLOT]], base=0,
                           channel_multiplier=0,
                           allow_small_or_imprecise_dtypes=True)
            sb_ident = st_pool.tile([P, P], f, name="sb_ident")
            make_identity(nc, sb_ident[:])

            import itertools
            for rep, layer in itertools.product(range(repeat), range(2)):
                src_t = xfull if layer == 0 else h1all
                gidx_t = gidx1 if layer == 0 else gidx2
                dstl_t = dstl1 if layer == 0 else dstl2
                wrel = sb_w1relT if layer == 0 else sb_w2relT
                wroot = sb_w1rootT if layer == 0 else sb_w2rootT
                bias = sb_b1 if layer == 0 else sb_b2
                wins = wins1 if layer == 0 else wins2

                for sg in range(cfg.supers):
                    gi_sb = gst_pool.tile([P, K * CW], I16, name="gi_sb")
                    nc.sync.dma_start(
                        out=gi_sb[:],
                        in_=gidx_t[:, sg * K * CW:(sg + 1) * K * CW])
                    dl_sb = gst_pool.tile([P, SGCH], f, name="dl_sb")
                    nc.sync.dma_start(
                        out=dl_sb[:],
                        in_=dstl_t[:, sg * SGCH:(sg + 1) * SGCH])
                    xg = xg_pool.tile([P, SGCH, D], f, name="xg")
                    selr = selr_pool.tile([P, K, SGR, BN], f, name="selr")
                    for k in range(K):
                        base, win = wins[k]
                        if skip_gather:
                            nc.vector.memset(xg[:, k * SGA:(k + 1) * SGA, :],
                                             0.0)
                        else:
                            nc.gpsimd.dma_gather(
                                out_ap=xg[:, k * SGA:(k + 1) * SGA, :],
                                in_ap=src_t[base:base + win, :],
                                idxs_ap=gi_sb[:, k * CW:(k + 1) * CW],
                                num_idxs=NIDX,
                                num_idxs_reg=NIDX,
                                elem_size=D,
                                single_packet=False,
                            )
                        nc.vector.tensor_tensor(
                            out=selr[:, k],
                            in0=sb_iota[:, :BN].unsqueeze(1)
                                .broadcast_to([P, SGR, BN]),
                            in1=dl_sb[:, k * SGA:k * SGA + SGR]
                                .unsqueeze(-1).broadcast_to([P, SGR, BN]),
                            op=mybir.AluOpType.is_equal,
                        )
                    for gl in range(SUPER):
                        if only_gather:
                            continue
                        g = sg * SUPER + gl
                        selo = selo_pool.tile([P, NOV, K, GSLOT], f,
                                              name="selo")
                        for o in range(NOV):
                            nc.vector.tensor_tensor(
                                out=selo[:, o],
                                in0=sb_iota[:, gl * GSLOT:(gl + 1) * GSLOT]
                                    .unsqueeze(1).broadcast_to([P, K, GSLOT]),
                                in1=dl_sb[:].rearrange("p (k j) -> p k j",
                                                       j=SGA)
                                    [:, :, SGR + o].unsqueeze(-1)
                                    .broadcast_to([P, K, GSLOT]),
                                op=mybir.AluOpType.is_equal,
                            )
                        pagg = pagg_pool.tile([D, GSLOT], f, name="pagg")
                        # full-region start first, then pure accumulation
                        nc.tensor.matmul(
                            out=pagg[:], lhsT=xg[:, SGR, :],
                            rhs=selo[:, 0, 0, :], start=True, stop=False,
                            skip_group_check=True)
                        for b in range(GB):
                            for k in range(K):
                                lc = k * SGA + gl * GB + b
                                nc.tensor.matmul(
                                    out=pagg[:, b * BN:(b + 1) * BN],
                                    lhsT=xg[:, lc, :],
                                    rhs=selr[:, k, gl * GB + b, :],
                                    start=False, stop=False,
                                    skip_group_check=True,
                                )
                        for o in range(NOV):
                            for k in range(K):
                                if o == 0 and k == 0:
                                    continue
                                nc.tensor.matmul(
                                    out=pagg[:],
                                    lhsT=xg[:, k * SGA + SGR + o, :],
                                    rhs=selo[:, o, k, :],
                                    start=False,
                                    stop=(o == NOV - 1 and k == K - 1),
                                    skip_group_check=True,
                                )
                        aggT = dr_pool.tile([D, GSLOT], f, name="aggT")
                        nc.vector.tensor_copy(out=aggT[:], in_=pagg[:])
                        if debug_dump and layer == 0 and sg == 0 and gl == 0:
                            nc.sync.dma_start(out=xgdbg[:], in_=xg[:])
                            nc.sync.dma_start(out=aggdbg[:], in_=aggT[:])
                        root_rhs = root_pool.tile([D, GSLOT], f, name="rootst")
                        rsrc = xpermT if layer == 0 else h1ownT
                        nc.sync.dma_start(
                            out=root_rhs[:],
                            in_=rsrc[:, g * GSLOT:(g + 1) * GSLOT])
                        ph = ph_pool.tile([D, GSLOT], f, name="ph")
                        nc.tensor.matmul(out=ph[:], lhsT=wrel[:], rhs=aggT[:],
                                         start=True, stop=False)
                        nc.tensor.matmul(out=ph[:], lhsT=wroot[:],
                                         rhs=root_rhs[:], start=False,
                                         stop=True)
                        if layer == 0:
                            hsl = dr_pool.tile([D, GSLOT], f, name="hsl")
                            nc.scalar.activation(out=hsl[:], in_=ph[:],
                                                 func=Relu, bias=bias[:, :1])
                            nc.sync.dma_start(
                                out=h1ownT[:, g * GSLOT:(g + 1) * GSLOT],
                                in_=hsl[:])
                            hr = dr_pool.tile([P, GSLOT // P, D], f, name="hr")
                            for q in range(GSLOT // P):
                                ptr = pmisc_pool.tile([P, D], f, name="ptr",
                                                      tag="pmisc")
                                nc.tensor.transpose(
                                    out=ptr[:],
                                    in_=hsl[:, q * P:(q + 1) * P],
                                    identity=sb_ident[:D, :D])
                                nc.vector.tensor_copy(out=hr[:, q, :],
                                                      in_=ptr[:])
                            nc.sync.dma_start(
                                out=h1own[g * GSLOT:(g + 1) * GSLOT, :]
                                    .rearrange("(q p) d -> p q d", p=P),
                                in_=hr[:])
                        else:
                            h2T = dr_pool.tile([D, GSLOT], f, name="h2T")
                            nc.scalar.activation(out=h2T[:], in_=ph[:],
                                                 func=Relu, bias=bias[:, :1])
                            po = pmisc_pool.tile([3, GSLOT], f, name="po",
                                                 tag="pmisc")
                            nc.tensor.matmul(out=po[:], lhsT=sb_wlinT[:],
                                             rhs=h2T[:], start=True, stop=True)
                            ot = out_pool.tile([3, GSLOT], f, name="ot")
                            nc.vector.tensor_scalar(
                                out=ot[:], in0=po[:], scalar1=sb_blin[:, :1],
                                scalar2=None, op0=mybir.AluOpType.add)
                            nc.sync.dma_start(
                                out=outT[:, g * GSLOT:(g + 1) * GSLOT],
                                in_=ot[:])

                if layer == 0 and not (skip_collective or only_gather):
                    nc.gpsimd.collective_compute(
                        "AllGather", mybir.AluOpType.bypass,
                        replica_groups=[list(range(cfg.n_cores))],
                        ins=[h1own[:]], outs=[h1all[:]])
                    if debug_dump:
                        nc.sync.dma_start(out=h1dbg[:], in_=h1own[:])
                        nc.sync.dma_start(out=h1alldbg[:], in_=h1all[:])

            if only_gather:
                nc.vector.memset(sb_iota[:], 0.0)
                nc.sync.dma_start(out=outT[:, :GSLOT], in_=sb_iota[:3, :])

    nc.compile()
    return nc


# ------------------------------------------------------------------ harness

def kernel(**inputs):
    """Full-input entry point: shards across 8 TRN2 cores, runs the Bass
    kernel via run_bass_kernel_spmd, returns the full [N, 3] float32 output."""
    from concourse.bass_utils import run_bass_kernel_spmd

    np_in = {k: np.asarray(v) for k, v in inputs.items()}
    cfg, in_maps, meta = prepare(
        np_in["x"], np_in["edge_index"],
        np_in["W1_rel"], np_in["b1_rel"], np_in["W1_root"],
        np_in["W2_rel"], np_in["b2_rel"], np_in["W2_root"],
        np_in["W_lin"], np_in["b_lin"], n_cores=8)
    nc = build_program(cfg)
    r = run_bass_kernel_spmd(nc, in_maps, core_ids=list(range(8)))
    return unshard(r.results, cfg, meta)
